# revision 8
# baseline (speedup 1.0000x reference)
"""Blockwise 2D DCT (out = C @ x @ C^T per 8x8 block) on 8 trn2 NeuronCores.

Strategy per core (data-parallel over leading batch dim, 16 batches/core):
  - View the core's shard as 16 contiguous 1 MiB chunks [128, 2048] fp32
    (fine-grained so the DMA/compute/store pipeline has short edges).
  - Per 128x128 sub-tile (256 blocks; one block = 64 contiguous floats in the
    free dim), in groups of 8 sharing two PSUM banks:
      1. PE transpose        -> pst[(e,q), m] in PSUM   (fp32, 2 cyc/row)
      2. DVE copy pst -> xt  (PSUM -> SBUF)
      3. PE matmul: stationary = xt, moving = BD = blockdiag(kron(C,C)^T x2).
         Output lands directly in natural block layout [m, (e, i*8+l)].
      4. DVE copy psm -> yout (PSUM -> SBUF), then contiguous 2 MiB store.
  - All HBM traffic is fully contiguous 2 MiB DMAs both directions.

TRN2 constraint honored throughout: every engine instruction can carry at
most ONE semaphore wait. All PSUM evacuations run on DVE so PE's data
dependency and its PSUM WAR dependency share one semaphore; two PE warm-up
transposes absorb the one-time const/DMA syncs; a tiny DVE "touch" per
mega-tile absorbs the store-DMA WAR so real copies never need two waits.
"""

import numpy as np

P = 128
COLS = 2048           # chunk free dim -> 1 MiB per chunk
MEGA = 16             # chunks per core
N_CORES = 8
GROUP = 8             # sub-tiles per chunk (2 PSUM banks per batch)
NGRP = COLS // (P * GROUP)   # 2 groups per chunk

_CACHE = {}


def _build_nc():
    import concourse.bass as bass
    import concourse.bacc as bacc
    import concourse.mybir as mybir
    import concourse.tile as tile
    from concourse.masks import make_identity

    f32 = mybir.dt.float32
    nc = bacc.Bacc()
    x_dram = nc.dram_tensor("x", [MEGA, P, COLS], f32, kind="ExternalInput")
    bd_dram = nc.dram_tensor("bd", [P, P], f32, kind="ExternalInput")
    y_dram = nc.dram_tensor("y", [MEGA, P, COLS], f32, kind="ExternalOutput")

    with tile.TileContext(nc) as tc:
        with (
            tc.tile_pool(name="consts", bufs=1) as consts,
            tc.tile_pool(name="xin", bufs=2) as xin_pool,
            tc.tile_pool(name="xt", bufs=3) as xt_pool,
            tc.tile_pool(name="yout", bufs=4) as yout_pool,
            tc.tile_pool(name="ps_t", bufs=2, space=bass.MemorySpace.PSUM) as ps_t_pool,
            tc.tile_pool(name="ps_m", bufs=2, space=bass.MemorySpace.PSUM) as ps_m_pool,
        ):
            ident = consts.tile([P, P], f32)
            make_identity(nc, ident[:])
            bdt = consts.tile([P, P], f32)
            nc.sync.dma_start(out=bdt[:], in_=bd_dram[:])

            for t in range(MEGA):
                xin = xin_pool.tile([P, COLS], f32)
                nc.sync.dma_start(out=xin[:], in_=x_dram[t])
                yout = yout_pool.tile([P, COLS], f32)
                for g in range(NGRP):
                    pst = ps_t_pool.tile([P, P * GROUP], f32)
                    for i in range(GROUP):
                        c = g * GROUP + i
                        nc.tensor.transpose(
                            pst[:, i * P:(i + 1) * P],
                            xin[:, c * P:(c + 1) * P],
                            ident[:],
                        )
                    xt = xt_pool.tile([P, P * GROUP], f32)
                    nc.vector.tensor_copy(xt[:], pst[:])
                    psm = ps_m_pool.tile([P, P * GROUP], f32)
                    for i in range(GROUP):
                        nc.tensor.matmul(
                            psm[:, i * P:(i + 1) * P],
                            xt[:, i * P:(i + 1) * P],
                            bdt[:],
                            start=True,
                            stop=True,
                        )
                    # ScalarE evacuates the matmul bank; DVE handles the
                    # transpose bank — keeps both copy streams off each
                    # other's engine.
                    nc.scalar.copy(
                        yout[:, g * P * GROUP:(g + 1) * P * GROUP], psm[:]
                    )
                nc.sync.dma_start(out=y_dram[t], in_=yout[:])
    nc.finalize()
    return nc


def _get_nc():
    if "nc" not in _CACHE:
        _CACHE["nc"] = _build_nc()
    return _CACHE["nc"]


def _make_bd(C):
    # out[i*8+l] = sum_{j*8+k} Mkron[i*8+l, j*8+k] * x[j*8+k], Mkron = kron(C, C).
    # matmul computes out[m, f] = sum_r xt[r, m] * bd[r, f] with r = 64e+q,
    # f = 64e'+u  ->  bd = blockdiag(Mkron^T, Mkron^T).
    C = np.asarray(C, dtype=np.float32)
    mk = np.kron(C, C).astype(np.float32)          # [64, 64]
    bd = np.zeros((P, P), dtype=np.float32)
    bd[:64, :64] = mk.T
    bd[64:, 64:] = mk.T
    return bd


def run_shards(x, C, **spmd_kwargs):
    """Run the kernel on 8 cores. Returns (list of per-core out dicts, BassKernelResults)."""
    from concourse.bass_utils import run_bass_kernel_spmd

    x = np.ascontiguousarray(np.asarray(x, dtype=np.float32))
    assert x.shape == (128, 4096, 8, 8), x.shape
    bd = _make_bd(C)
    shards = x.reshape(N_CORES, MEGA, P, COLS)
    in_maps = [{"x": shards[c], "bd": bd} for c in range(N_CORES)]
    nc = _get_nc()
    res = run_bass_kernel_spmd(nc, in_maps, core_ids=list(range(N_CORES)), **spmd_kwargs)
    return res.results, res


def kernel(x, C):
    results, _ = run_shards(x, C)
    out = np.empty((N_CORES, MEGA, P, COLS), dtype=np.float32)
    for c in range(N_CORES):
        out[c] = results[c]["y"]
    return out.reshape(128, 4096, 8, 8)


# revision 12
# speedup vs baseline: 1.0732x; 1.0732x over previous
"""Blockwise 2D DCT (out = C @ x @ C^T per 8x8 block) on 8 trn2 NeuronCores.

Strategy per core (data-parallel over leading batch dim, 16 batches/core):
  - View the core's shard as 16 contiguous 1 MiB chunks [128, 2048] fp32
    (fine-grained so the DMA/compute/store pipeline has short edges).
  - Per 128x128 sub-tile (256 blocks; one block = 64 contiguous floats in the
    free dim), in groups of 8 sharing two PSUM banks:
      1. PE transpose        -> pst[(e,q), m] in PSUM   (fp32, 2 cyc/row)
      2. DVE copy pst -> xt  (PSUM -> SBUF)
      3. PE matmul: stationary = xt, moving = BD = blockdiag(kron(C,C)^T x2).
         Output lands directly in natural block layout [m, (e, i*8+l)].
      4. DVE copy psm -> yout (PSUM -> SBUF), then contiguous 2 MiB store.
  - All HBM traffic is fully contiguous 2 MiB DMAs both directions.

TRN2 constraint honored throughout: every engine instruction can carry at
most ONE semaphore wait. All PSUM evacuations run on DVE so PE's data
dependency and its PSUM WAR dependency share one semaphore; two PE warm-up
transposes absorb the one-time const/DMA syncs; a tiny DVE "touch" per
mega-tile absorbs the store-DMA WAR so real copies never need two waits.
"""

import numpy as np

P = 128
N_CORES = 8
TOTAL_COLS = 32768    # per-core free dim (16 MiB / 128 partitions / 4 B)
GROUP = 8             # sub-tiles per PSUM batch (2 banks)
# Chunk column sizes: small chunks at both edges so the first compute starts
# early and the last store drains fast; 1 MiB (2048-col) chunks in the middle.
CHUNK_COLS = [512, 512, 512, 512] + [2048] * 14 + [1024, 1024]
assert sum(CHUNK_COLS) == TOTAL_COLS

_CACHE = {}


def _build_nc():
    import concourse.bass as bass
    import concourse.bacc as bacc
    import concourse.mybir as mybir
    import concourse.tile as tile
    from concourse.masks import make_identity

    f32 = mybir.dt.float32
    nc = bacc.Bacc()
    x_dram = nc.dram_tensor("x", [P * TOTAL_COLS], f32, kind="ExternalInput")
    bd_dram = nc.dram_tensor("bd", [P, P], f32, kind="ExternalInput")
    y_dram = nc.dram_tensor("y", [P * TOTAL_COLS], f32, kind="ExternalOutput")

    with tile.TileContext(nc) as tc:
        with (
            tc.tile_pool(name="consts", bufs=1) as consts,
            tc.tile_pool(name="xin", bufs=4) as xin_pool,
            tc.tile_pool(name="xt", bufs=4) as xt_pool,
            tc.tile_pool(name="yout", bufs=4) as yout_pool,
            tc.tile_pool(name="ps_t", bufs=2, space=bass.MemorySpace.PSUM) as ps_t_pool,
            tc.tile_pool(name="ps_m", bufs=2, space=bass.MemorySpace.PSUM) as ps_m_pool,
        ):
            ident = consts.tile([P, P], f32)
            make_identity(nc, ident[:])
            bdt = consts.tile([P, P], f32)
            nc.sync.dma_start(out=bdt[:], in_=bd_dram[:])

            off = 0
            for cols in CHUNK_COLS:
                x_view = x_dram[off:off + P * cols].rearrange("(p c) -> p c", p=P)
                y_view = y_dram[off:off + P * cols].rearrange("(p c) -> p c", p=P)
                off += P * cols
                n_sub = cols // P
                groups = [
                    (g * GROUP, min(GROUP, n_sub - g * GROUP))
                    for g in range((n_sub + GROUP - 1) // GROUP)
                ]
                xin = xin_pool.tile([P, cols], f32, tag="xin")
                nc.sync.dma_start(out=xin[:], in_=x_view)
                yout = yout_pool.tile([P, cols], f32, tag="yout")
                # All transposes (+ DVE evacuations) for the chunk first, then
                # the matmul batches: PE never idles waiting for an xt copy —
                # it runs the next group's transposes instead.
                xts = []
                for c0, gsz in groups:
                    pst = ps_t_pool.tile([P, P * gsz], f32, tag="pst")
                    for i in range(gsz):
                        c = c0 + i
                        nc.tensor.transpose(
                            pst[:, i * P:(i + 1) * P],
                            xin[:, c * P:(c + 1) * P],
                            ident[:],
                        )
                    xt = xt_pool.tile([P, P * gsz], f32, tag="xt")
                    nc.vector.tensor_copy(xt[:], pst[:])
                    xts.append(xt)
                for (c0, gsz), xt in zip(groups, xts):
                    psm = ps_m_pool.tile([P, P * gsz], f32, tag="psm")
                    for i in range(gsz):
                        nc.tensor.matmul(
                            psm[:, i * P:(i + 1) * P],
                            xt[:, i * P:(i + 1) * P],
                            bdt[:],
                            start=True,
                            stop=True,
                        )
                    # ScalarE evacuates the matmul bank; DVE handles the
                    # transpose bank — keeps the two copy streams on
                    # separate engines.
                    nc.scalar.copy(yout[:, c0 * P:(c0 + gsz) * P], psm[:])
                nc.sync.dma_start(out=y_view, in_=yout[:])
    nc.finalize()
    return nc


def _get_nc():
    if "nc" not in _CACHE:
        _CACHE["nc"] = _build_nc()
    return _CACHE["nc"]


def _make_bd(C):
    # out[i*8+l] = sum_{j*8+k} Mkron[i*8+l, j*8+k] * x[j*8+k], Mkron = kron(C, C).
    # matmul computes out[m, f] = sum_r xt[r, m] * bd[r, f] with r = 64e+q,
    # f = 64e'+u  ->  bd = blockdiag(Mkron^T, Mkron^T).
    C = np.asarray(C, dtype=np.float32)
    mk = np.kron(C, C).astype(np.float32)          # [64, 64]
    bd = np.zeros((P, P), dtype=np.float32)
    bd[:64, :64] = mk.T
    bd[64:, 64:] = mk.T
    return bd


def run_shards(x, C, **spmd_kwargs):
    """Run the kernel on 8 cores. Returns (list of per-core out dicts, BassKernelResults)."""
    from concourse.bass_utils import run_bass_kernel_spmd

    x = np.ascontiguousarray(np.asarray(x, dtype=np.float32))
    assert x.shape == (128, 4096, 8, 8), x.shape
    bd = _make_bd(C)
    shards = x.reshape(N_CORES, P * TOTAL_COLS)
    in_maps = [{"x": shards[c], "bd": bd} for c in range(N_CORES)]
    nc = _get_nc()
    res = run_bass_kernel_spmd(nc, in_maps, core_ids=list(range(N_CORES)), **spmd_kwargs)
    return res.results, res


def kernel(x, C):
    results, _ = run_shards(x, C)
    out = np.empty((N_CORES, P * TOTAL_COLS), dtype=np.float32)
    for c in range(N_CORES):
        out[c] = results[c]["y"]
    return out.reshape(128, 4096, 8, 8)


# revision 14
# speedup vs baseline: 1.1497x; 1.0713x over previous
"""Blockwise 2D DCT (out = C @ x @ C^T per 8x8 block) on 8 trn2 NeuronCores.

Strategy per core (data-parallel over leading batch dim, 16 batches/core):
  - View the core's shard as 16 contiguous 1 MiB chunks [128, 2048] fp32
    (fine-grained so the DMA/compute/store pipeline has short edges).
  - Per 128x128 sub-tile (256 blocks; one block = 64 contiguous floats in the
    free dim), in groups of 8 sharing two PSUM banks:
      1. PE transpose        -> pst[(e,q), m] in PSUM   (fp32, 2 cyc/row)
      2. DVE copy pst -> xt  (PSUM -> SBUF)
      3. PE matmul: stationary = xt, moving = BD = blockdiag(kron(C,C)^T x2).
         Output lands directly in natural block layout [m, (e, i*8+l)].
      4. DVE copy psm -> yout (PSUM -> SBUF), then contiguous 2 MiB store.
  - All HBM traffic is fully contiguous 2 MiB DMAs both directions.

TRN2 constraint honored throughout: every engine instruction can carry at
most ONE semaphore wait. All PSUM evacuations run on DVE so PE's data
dependency and its PSUM WAR dependency share one semaphore; two PE warm-up
transposes absorb the one-time const/DMA syncs; a tiny DVE "touch" per
mega-tile absorbs the store-DMA WAR so real copies never need two waits.
"""

import numpy as np

P = 128
N_CORES = 8
TOTAL_COLS = 32768    # per-core free dim (16 MiB / 128 partitions / 4 B)
GROUP = 8             # sub-tiles per PSUM batch (2 banks)
# Chunk column sizes: small chunks at both edges so the first compute starts
# early and the last store drains fast; 1 MiB (2048-col) chunks in the middle.
CHUNK_COLS = [512, 512, 512, 512] + [2048] * 14 + [1024, 1024]
assert sum(CHUNK_COLS) == TOTAL_COLS

_CACHE = {}


def _build_nc():
    import concourse.bass as bass
    import concourse.bacc as bacc
    import concourse.mybir as mybir
    import concourse.tile as tile
    from concourse.masks import make_identity

    f32 = mybir.dt.float32
    nc = bacc.Bacc()
    x_dram = nc.dram_tensor("x", [P * TOTAL_COLS], f32, kind="ExternalInput")
    bd_dram = nc.dram_tensor("bd", [P, P], f32, kind="ExternalInput")
    y_dram = nc.dram_tensor("y", [P * TOTAL_COLS], f32, kind="ExternalOutput")

    with tile.TileContext(nc) as tc:
        with (
            tc.tile_pool(name="consts", bufs=1) as consts,
            tc.tile_pool(name="xin", bufs=4) as xin_pool,
            tc.tile_pool(name="xt", bufs=4) as xt_pool,
            tc.tile_pool(name="yout", bufs=4) as yout_pool,
            tc.tile_pool(name="ps_t", bufs=2, space=bass.MemorySpace.PSUM) as ps_t_pool,
            tc.tile_pool(name="ps_m", bufs=2, space=bass.MemorySpace.PSUM) as ps_m_pool,
        ):
            ident = consts.tile([P, P], f32)
            make_identity(nc, ident[:])
            bdt = consts.tile([P, P], f32)
            nc.sync.dma_start(out=bdt[:], in_=bd_dram[:])

            off = 0
            for cols in CHUNK_COLS:
                x_view = x_dram[off:off + P * cols].rearrange("(p c) -> p c", p=P)
                y_view = y_dram[off:off + P * cols].rearrange("(p c) -> p c", p=P)
                off += P * cols
                n_sub = cols // P
                groups = [
                    (g * GROUP, min(GROUP, n_sub - g * GROUP))
                    for g in range((n_sub + GROUP - 1) // GROUP)
                ]
                xin = xin_pool.tile([P, cols], f32, tag="xin")
                nc.sync.dma_start(out=xin[:], in_=x_view)
                yout = yout_pool.tile([P, cols], f32, tag="yout")
                # All transposes (+ DVE evacuations) for the chunk first, then
                # the matmul batches: PE never idles waiting for an xt copy —
                # it runs the next group's transposes instead.
                xts = []
                for c0, gsz in groups:
                    pst = ps_t_pool.tile([P, P * gsz], f32, tag="pst")
                    xt = xt_pool.tile([P, P * gsz], f32, tag="xt")
                    # Evacuate per half-group (one PSUM bank at a time) so the
                    # first matmuls of the group never wait on the full copy.
                    half = (gsz + 1) // 2
                    for i in range(gsz):
                        c = c0 + i
                        nc.tensor.transpose(
                            pst[:, i * P:(i + 1) * P],
                            xin[:, c * P:(c + 1) * P],
                            ident[:],
                        )
                        if i + 1 == half:
                            nc.vector.tensor_copy(
                                xt[:, :half * P], pst[:, :half * P]
                            )
                    if gsz > half:
                        nc.vector.tensor_copy(
                            xt[:, half * P:gsz * P], pst[:, half * P:gsz * P]
                        )
                    xts.append(xt)
                for (c0, gsz), xt in zip(groups, xts):
                    psm = ps_m_pool.tile([P, P * gsz], f32, tag="psm")
                    for i in range(gsz):
                        nc.tensor.matmul(
                            psm[:, i * P:(i + 1) * P],
                            xt[:, i * P:(i + 1) * P],
                            bdt[:],
                            start=True,
                            stop=True,
                        )
                    # ScalarE evacuates the matmul bank; DVE handles the
                    # transpose bank — keeps the two copy streams on
                    # separate engines.
                    nc.scalar.copy(yout[:, c0 * P:(c0 + gsz) * P], psm[:])
                # Store via the ScalarE HWDGE ring: it directly follows the
                # last yout copy on the same engine (no semaphore wait), and
                # keeps the Sync ring free for loads — a store waiting on its
                # copy would otherwise head-of-line-block the next loads.
                nc.scalar.dma_start(out=y_view, in_=yout[:])
    nc.finalize()
    return nc


def _get_nc():
    if "nc" not in _CACHE:
        _CACHE["nc"] = _build_nc()
    return _CACHE["nc"]


def _make_bd(C):
    # out[i*8+l] = sum_{j*8+k} Mkron[i*8+l, j*8+k] * x[j*8+k], Mkron = kron(C, C).
    # matmul computes out[m, f] = sum_r xt[r, m] * bd[r, f] with r = 64e+q,
    # f = 64e'+u  ->  bd = blockdiag(Mkron^T, Mkron^T).
    C = np.asarray(C, dtype=np.float32)
    mk = np.kron(C, C).astype(np.float32)          # [64, 64]
    bd = np.zeros((P, P), dtype=np.float32)
    bd[:64, :64] = mk.T
    bd[64:, 64:] = mk.T
    return bd


def run_shards(x, C, **spmd_kwargs):
    """Run the kernel on 8 cores. Returns (list of per-core out dicts, BassKernelResults)."""
    from concourse.bass_utils import run_bass_kernel_spmd

    x = np.ascontiguousarray(np.asarray(x, dtype=np.float32))
    assert x.shape == (128, 4096, 8, 8), x.shape
    bd = _make_bd(C)
    shards = x.reshape(N_CORES, P * TOTAL_COLS)
    in_maps = [{"x": shards[c], "bd": bd} for c in range(N_CORES)]
    nc = _get_nc()
    res = run_bass_kernel_spmd(nc, in_maps, core_ids=list(range(N_CORES)), **spmd_kwargs)
    return res.results, res


def kernel(x, C):
    results, _ = run_shards(x, C)
    out = np.empty((N_CORES, P * TOTAL_COLS), dtype=np.float32)
    for c in range(N_CORES):
        out[c] = results[c]["y"]
    return out.reshape(128, 4096, 8, 8)
